# revision 12
# baseline (speedup 1.0000x reference)
"""Trainium2 Bass kernel for nn_DAWN_87677462380612 (moe_routing).

Sharding: 8 cores = 2 batches x 4 sequence chunks of 256 tokens.
Per layer: each core computes LN1/Q/K/V for its own 256 tokens, AllGathers
(K feature-major, V token-major) within its batch group of 4 cores, runs
causal attention for its queries against all 1024 keys (transposed-score
formulation: S_T[k,q] so the exp'd probs serve directly as AV's lhsT, with
a ones-column in V producing the softmax denominator), computes routing
scores, exact top-16 via vector max8 + match_replace + max8 (threshold
trick turns the top-k gather into dense masked-softmax GEMMs), then the
basis-coordinate MLP. Final LN locally -> AllGather(all 8, transposed) ->
vocab-parallel logits GEMM (4000 vocab columns per core) in float32r.

All trunk GEMMs run in fp32 (the reference top-16 selection has score gaps
down to 2.3e-6; bf16 anywhere flips selections and produces O(1) logit
errors). Softmax exp skips max-subtraction (|logit| <= 1.3). Gelu is exact
via the Erf LUT with the 0.5 factor folded into dw_w host-side.
"""
import ml_dtypes
import numpy as np

import concourse.bacc as bacc
import concourse.bass as bass
import concourse.mybir as mybir
import concourse.tile as tile

F32 = mybir.dt.float32
F32R = mybir.dt.float32r
BF16 = mybir.dt.bfloat16
I16 = mybir.dt.int16
AF = mybir.ActivationFunctionType
ALU = mybir.AluOpType
AX = mybir.AxisListType

B, S, V, D, DF, L, H, NN, K, NB, R = 2, 1024, 32000, 512, 2048, 4, 8, 512, 16, 8, 64
NCORES, GROUP, CHUNK = 8, 4, 256
VS = V // NCORES          # 4000
DH = D // H               # 64
NT = CHUNK // 128         # 2 token tiles per core
KC = S // 128             # 8 key chunks
DC = D // 128             # 4 feature chunks
FC = DF // 128            # 16
NBR = NB * R              # 512
VCH = 500                 # vocab chunk
VHALF = VS // 2           # 2000 (embT loaded in halves)
EPS = 1e-5

SIM = False
TRACE = False
DEBUG_TAPS = False
LAST_EXEC_NS = None

_cache = {}


def _rsqrt(nc, small, v):
    """Accurate 1/sqrt(v): sqrt LUT + reciprocal + 2 Newton steps."""
    sq = small.tile([128, 1], F32, tag="rs0")
    nc.scalar.sqrt(sq[:], v[:])
    r = small.tile([128, 1], F32, tag="rs1")
    nc.vector.reciprocal(r[:], sq[:])
    for i in range(2):
        a = small.tile([128, 1], F32, tag=f"rs2_{i}")
        nc.vector.tensor_mul(a[:], r[:], r[:])
        bq = small.tile([128, 1], F32, tag=f"rs3_{i}")
        nc.vector.tensor_mul(bq[:], a[:], v[:])
        d = small.tile([128, 1], F32, tag=f"rs4_{i}")
        nc.vector.tensor_scalar(d[:], bq[:], -0.5, 1.5, op0=ALU.mult, op1=ALU.add)
        r2 = small.tile([128, 1], F32, tag=f"rs5_{i}")
        nc.vector.tensor_mul(r2[:], r[:], d[:])
        r = r2
    return r


def _layernorm(nc, pools, x_ap):
    """LN over free axis (D=512), unit scale / zero bias. Returns [128, D]."""
    act, small = pools["act"], pools["small"]
    s = small.tile([128, 1], F32, tag="ln_s")
    nc.vector.reduce_sum(s[:], x_ap, axis=AX.X)
    nm = small.tile([128, 1], F32, tag="ln_nm")
    nc.vector.tensor_scalar_mul(nm[:], s[:], -1.0 / D)
    ct = act.tile([128, D], F32, tag="ln_c")
    nc.vector.tensor_scalar_add(ct[:], x_ap, nm[:])
    sq = act.tile([128, D], F32, tag="ctx")  # junk buffer, reuse ctx slot
    ss = small.tile([128, 1], F32, tag="ln_ss")
    nc.scalar.activation(sq[:], ct[:], AF.Square, accum_out=ss[:])
    v = small.tile([128, 1], F32, tag="ln_v")
    nc.vector.tensor_scalar(v[:], ss[:], 1.0 / D, EPS, op0=ALU.mult, op1=ALU.add)
    r = _rsqrt(nc, small, v)
    out = act.tile([128, D], F32, tag="ln_o")
    nc.vector.tensor_scalar_mul(out[:], ct[:], r[:])
    return out


def _build():
    if "built" in _cache:
        return _cache["built"]
    import concourse.tile_utils as tile_utils
    tile_utils.max_sbuf_usage = 207 * 1024  # cayman: 208KB usable/partition

    nc = bacc.Bacc("TRN2", target_bir_lowering=False, debug=False,
                   num_devices=NCORES)

    d_idx = nc.dram_tensor("idx16", [128, 16], I16, kind="ExternalInput")
    d_emb = nc.dram_tensor("emb", [V, D], F32, kind="ExternalInput")
    d_pos = nc.dram_tensor("pos", [128, NT, D], F32, kind="ExternalInput")
    d_mask = nc.dram_tensor("mask01", [128, KC, CHUNK], F32, kind="ExternalInput")
    d_ident = nc.dram_tensor("ident", [128, 128], F32, kind="ExternalInput")
    d_qw = nc.dram_tensor("qw", [L, D, D], F32, kind="ExternalInput")
    d_kw = nc.dram_tensor("kw", [L, D, D], F32, kind="ExternalInput")
    d_vw = nc.dram_tensor("vw", [L, D, D], F32, kind="ExternalInput")
    d_nembT = nc.dram_tensor("nembT", [L, D, NN], F32, kind="ExternalInput")
    d_nemb = nc.dram_tensor("nemb", [L, NN, D], F32, kind="ExternalInput")
    d_coords = nc.dram_tensor("coords", [L, NN, NB], F32, kind="ExternalInput")
    d_gw1 = nc.dram_tensor("gw1", [L, D, 2], F32, kind="ExternalInput")
    d_gw2 = nc.dram_tensor("gw2", [L, D, 2], F32, kind="ExternalInput")
    d_aflat = nc.dram_tensor("aflat", [L, D, NBR], F32, kind="ExternalInput")
    d_bflat = nc.dram_tensor("bflat", [L, NBR, DF], F32, kind="ExternalInput")
    d_dwwh = nc.dram_tensor("dwwh", [L, DF, D], F32, kind="ExternalInput")
    d_embT = nc.dram_tensor("embTs", [D, VS], BF16, kind="ExternalInput")
    d_out = nc.dram_tensor("out", [B * S, VS], F32, kind="ExternalOutput")
    d_tap = None
    if DEBUG_TAPS:
        d_tap = nc.dram_tensor("tap", [16, 128, NN], F32, kind="ExternalOutput")

    kv_in = nc.dram_tensor("kv_in", [S, CHUNK], F32)
    kv_out = nc.dram_tensor("kv_out", [GROUP * S, CHUNK], F32)
    ag2_in = nc.dram_tensor("ag2_in", [D, CHUNK], BF16)
    ag2_out = nc.dram_tensor("ag2_out", [NCORES * D, CHUNK], BF16,
                             addr_space="Shared")
    KV_GROUPS = [[0, 1, 2, 3], [4, 5, 6, 7]]
    ALL_GROUP = [list(range(NCORES))]

    with tile.TileContext(nc) as tc:
        with (
            tc.tile_pool(name="persist", bufs=1) as persist,
            tc.tile_pool(name="wts", bufs=1) as wts,
            tc.tile_pool(name="act", bufs=1) as act,
            tc.tile_pool(name="actD", bufs=2) as actD,
            tc.tile_pool(name="attp", bufs=3) as attp,
            tc.tile_pool(name="mid1", bufs=1) as mid1,
            tc.tile_pool(name="mid2", bufs=2) as mid2,
            tc.tile_pool(name="small", bufs=2) as small,
            tc.tile_pool(name="psA", bufs=2, space="PSUM") as psA,
            tc.tile_pool(name="psB", bufs=2, space="PSUM") as psB,
            tc.tile_pool(name="psC", bufs=1, space="PSUM") as psC,
        ):
            pools = {"act": act, "small": small}

            def transpose_to(src_ap, dst_ap, use_act):
                pt = psB.tile([128, 128], F32, tag="tr")
                nc.tensor.transpose(pt[:], src_ap, ident[:])
                if use_act:
                    nc.scalar.copy(dst_ap, pt[:])
                else:
                    nc.vector.tensor_copy(dst_ap, pt[:])

            ident = persist.tile([128, 128], F32, tag="ident")
            nc.sync.dma_start(ident[:], d_ident.ap())
            mask = persist.tile([128, KC, CHUNK], F32, tag="mask")
            nc.sync.dma_start(mask[:], d_mask.ap())

            # ---- embedding gather + positional ----
            idx = persist.tile([128, 16], I16, tag="idx")
            nc.sync.dma_start(idx[:], d_idx.ap())
            xg = act.tile([128, NT, D], F32, tag="qT")
            nc.gpsimd.dma_gather(xg[:], d_emb.ap(), idx[:], num_idxs=CHUNK,
                                 num_idxs_reg=CHUNK, elem_size=D)
            pos = act.tile([128, NT, D], F32, tag="kTo")
            nc.sync.dma_start(pos[:], d_pos.ap())
            x_sb = []
            for t in range(NT):
                xt = persist.tile([128, D], F32, tag=f"x{t}")
                nc.vector.tensor_add(xt[:], xg[:, t, :], pos[:, t, :])
                x_sb.append(xt)

            for l in range(L):
                # ---- layer weights (single-buffered slots, streamed) ----
                qw_sb = wts.tile([128, DC, D], F32, tag="qw")
                kw_sb = wts.tile([128, DC, D], F32, tag="kw")
                vw_sb = wts.tile([128, DC, D], F32, tag="vw")
                nembT_sb = wts.tile([128, DC, NN], F32, tag="nembT")
                nemb_sb = wts.tile([128, DC, D], F32, tag="nemb")
                coords_sb = wts.tile([128, DC, NB], F32, tag="coords")
                gw1_sb = wts.tile([128, DC, 2], F32, tag="gw1")
                gw2_sb = wts.tile([128, DC, 2], F32, tag="gw2")
                aflat_sb = wts.tile([128, DC, NBR], F32, tag="aflat")
                bflat_sb = wts.tile([128, DC, DF], F32, tag="bflat")
                for c in range(DC):
                    rows = slice(c * 128, (c + 1) * 128)
                    nc.sync.dma_start(qw_sb[:, c, :], d_qw.ap()[l, rows, :])
                    nc.sync.dma_start(kw_sb[:, c, :], d_kw.ap()[l, rows, :])
                    nc.sync.dma_start(vw_sb[:, c, :], d_vw.ap()[l, rows, :])
                    nc.sync.dma_start(nembT_sb[:, c, :], d_nembT.ap()[l, rows, :])
                    nc.sync.dma_start(nemb_sb[:, c, :], d_nemb.ap()[l, rows, :])
                    nc.sync.dma_start(coords_sb[:, c, :], d_coords.ap()[l, rows, :])
                    nc.sync.dma_start(gw1_sb[:, c, :], d_gw1.ap()[l, rows, :])
                    nc.sync.dma_start(gw2_sb[:, c, :], d_gw2.ap()[l, rows, :])
                    nc.sync.dma_start(aflat_sb[:, c, :], d_aflat.ap()[l, rows, :])
                    nc.sync.dma_start(bflat_sb[:, c, :], d_bflat.ap()[l, rows, :])
                dwwh_sb = wts.tile([128, FC, D], F32, tag="dwwh")
                for c in range(FC):
                    nc.sync.dma_start(dwwh_sb[:, c, :],
                                      d_dwwh.ap()[l, c * 128:(c + 1) * 128, :])

                # ---- LN1 + feature-major transpose ----
                n1T = act.tile([128, DC, CHUNK], F32, tag="n1T")
                for t in range(NT):
                    n1_t = _layernorm(nc, pools, x_sb[t][:])
                    if DEBUG_TAPS and l == 0:
                        nc.sync.dma_start(d_tap.ap()[0 + t, :, :], n1_t[:])
                    for c in range(DC):
                        transpose_to(n1_t[:, c * 128:(c + 1) * 128],
                                     n1T[:, c, t * 128:(t + 1) * 128],
                                     use_act=(c % 2 == 0))

                # ---- Q,K feature-major / V token-major (own tokens) ----
                qT = act.tile([128, DC, CHUNK], F32, tag="qT")
                kTo = act.tile([128, DC, CHUNK], F32, tag="kTo")
                for o in range(DC):
                    for wsb, dst in ((qw_sb, qT), (kw_sb, kTo)):
                        pp = psA.tile([128, CHUNK], F32, tag="mm1")
                        for c in range(DC):
                            nc.tensor.matmul(pp[:],
                                             wsb[:, c, o * 128:(o + 1) * 128],
                                             n1T[:, c, :],
                                             start=(c == 0), stop=(c == DC - 1))
                        nc.scalar.copy(dst[:, o, :], pp[:])
                for c in range(DC):
                    nc.sync.dma_start(kv_in.ap()[c * 128:(c + 1) * 128, :],
                                      kTo[:, c, :])
                for t in range(NT):
                    pp = psA.tile([128, D], F32, tag="mm1")
                    for c in range(DC):
                        nc.tensor.matmul(pp[:], n1T[:, c, t * 128:(t + 1) * 128],
                                         vw_sb[:, c, :],
                                         start=(c == 0), stop=(c == DC - 1))
                    vt = actD.tile([128, D], F32, tag="v_own")
                    nc.vector.tensor_copy(vt[:], pp[:])
                    dst = kv_in.ap()[D + t * 256:D + (t + 1) * 256, :].rearrange(
                        "(p two) f -> p (two f)", two=2)
                    nc.sync.dma_start(dst, vt[:])
                nc.gpsimd.collective_compute(
                    "AllGather", ALU.bypass, replica_groups=KV_GROUPS,
                    ins=[kv_in.ap().opt()], outs=[kv_out.ap().opt()])

                # ---- attention: stream key-chunks ----
                # Per kc: single-shot AV matmuls into a 4-bank psum tile
                # (512B slot per (h,t) so each write stays inside one bank;
                # no cross-kc psum accumulation -- the 2KB zero-region start
                # semantics would discard sibling slots' data). DVE folds the
                # partials into an SBUF accumulator region-by-region so the
                # adds pipeline against the PE.
                ctxacc = act.tile([128, H, NT, DH + 1], F32, tag="ctxacc")
                for kc in range(KC):
                    r, th = divmod(kc, 2)
                    KTc = actD.tile([128, DC, 128], F32, tag="KTc")
                    for c in range(DC):
                        nc.sync.dma_start(
                            KTc[:, c, :],
                            kv_out.ap()[r * S + c * 128:r * S + (c + 1) * 128,
                                        th * 128:(th + 1) * 128])
                    Vgc = actD.tile([128, H, DH + 1], F32, tag="Vgc")
                    src = kv_out.ap()[r * S + D + th * 256:
                                      r * S + D + (th + 1) * 256, :]
                    src = src.rearrange("(p two) f -> p (two f)", two=2)
                    src = src.rearrange("p (h dh) -> p h dh", h=H)
                    nc.sync.dma_start(Vgc[:, :, 0:DH], src)
                    nc.vector.memset(Vgc[:, :, DH], 1.0)
                    ctx_ps = psC.tile([128, H, NT, 128], F32, tag="big")
                    for h in range(H):
                        po = (h % 2) * 64
                        ch = h // 2
                        st = psB.tile([128, CHUNK], F32, tag="tr")
                        nc.tensor.matmul(st[:], KTc[po:po + DH, ch, :],
                                         qT[po:po + DH, ch, :],
                                         start=True, stop=True)
                        pe = attp.tile([128, CHUNK], F32, tag="pe")
                        nc.scalar.activation(pe[:], st[:], AF.Exp, scale=0.125)
                        pm = attp.tile([128, CHUNK], F32, tag="pm")
                        nc.vector.tensor_mul(pm[:], pe[:], mask[:, kc, :])
                        for t in range(NT):
                            nc.tensor.matmul(
                                ctx_ps[:, h, t, 0:DH + 1],
                                pm[:, t * 128:(t + 1) * 128],
                                Vgc[:, h, :],
                                start=True, stop=True,
                                skip_group_check=True)
                        if h % 2 == 1:
                            hs = slice(h - 1, h + 1)
                            if kc == 0:
                                nc.vector.tensor_copy(
                                    ctxacc[:, hs, :, :],
                                    ctx_ps[:, hs, :, 0:DH + 1])
                            else:
                                nc.vector.tensor_add(
                                    ctxacc[:, hs, :, :], ctxacc[:, hs, :, :],
                                    ctx_ps[:, hs, :, 0:DH + 1])

                # ---- ctx normalize + transpose ----
                ctxnT = act.tile([128, DC, CHUNK], F32, tag="qT")
                for t in range(NT):
                    cx = act.tile([128, H, DH], F32, tag="ctx")
                    nc.scalar.copy(cx[:], ctxacc[:, :, t, 0:DH])
                    z = small.tile([128, H], F32, tag="z")
                    nc.vector.tensor_copy(z[:], ctxacc[:, :, t, DH])
                    rz = small.tile([128, H], F32, tag="rz")
                    nc.vector.reciprocal(rz[:], z[:])
                    cn = act.tile([128, H, DH], F32, tag="ctxn")
                    nc.vector.tensor_tensor(
                        cn[:], cx[:],
                        rz[:, :, None].broadcast_to([128, H, DH]), op=ALU.mult)
                    cnf = cn[:].rearrange("p h dh -> p (h dh)")
                    if DEBUG_TAPS and l == 0:
                        nc.sync.dma_start(d_tap.ap()[2 + t, :, :], cnf)
                    for c in range(DC):
                        transpose_to(cnf[:, c * 128:(c + 1) * 128],
                                     ctxnT[:, c, t * 128:(t + 1) * 128],
                                     use_act=(c % 2 == 1))

                # ---- routing + MLP per token tile ----
                for t in range(NT):
                    tsl = slice(t * 128, (t + 1) * 128)
                    ts_ps = psA.tile([128, NN], F32, tag="mm1")
                    for c in range(DC):
                        nc.tensor.matmul(ts_ps[:], n1T[:, c, tsl],
                                         nembT_sb[:, c, :],
                                         start=(c == 0), stop=(c == DC - 1))
                    cs_ps = psA.tile([128, NN], F32, tag="mm1")
                    for c in range(DC):
                        nc.tensor.matmul(cs_ps[:], ctxnT[:, c, tsl],
                                         nembT_sb[:, c, :],
                                         start=(c == 0), stop=(c == DC - 1))
                    gz_ps = psC.tile([128, 2], F32, tag="big")
                    for c in range(DC):
                        nc.tensor.matmul(gz_ps[:], n1T[:, c, tsl],
                                         gw1_sb[:, c, :],
                                         start=(c == 0), stop=False)
                        nc.tensor.matmul(gz_ps[:], ctxnT[:, c, tsl],
                                         gw2_sb[:, c, :],
                                         start=False, stop=(c == DC - 1))
                    gz_sb = small.tile([128, 2], F32, tag="gz_sb")
                    nc.scalar.copy(gz_sb[:], gz_ps[:])
                    zd = small.tile([128, 1], F32, tag="zd")
                    nc.vector.tensor_sub(zd[:], gz_sb[:, 1:2], gz_sb[:, 0:1])
                    en = small.tile([128, 1], F32, tag="en")
                    nc.scalar.activation(en[:], zd[:], AF.Exp, scale=-1.0)
                    sden = small.tile([128, 1], F32, tag="sden")
                    nc.vector.tensor_scalar_add(sden[:], en[:], 1.0)
                    g1 = small.tile([128, 1], F32, tag="g1")
                    nc.vector.reciprocal(g1[:], sden[:])
                    g0 = small.tile([128, 1], F32, tag="g0")
                    nc.vector.tensor_mul(g0[:], en[:], g1[:])

                    t1 = mid2.tile([128, NN], F32, tag="t1")
                    nc.vector.tensor_scalar_mul(t1[:], cs_ps[:], g1[:])
                    sc = mid2.tile([128, NN], F32, tag="sc")
                    nc.vector.scalar_tensor_tensor(sc[:], ts_ps[:], g0[:], t1[:],
                                                   op0=ALU.mult, op1=ALU.add)
                    if DEBUG_TAPS and l == 0:
                        nc.sync.dma_start(d_tap.ap()[4 + t, :, :], sc[:])
                    m8a = small.tile([128, 8], F32, tag="m8a")
                    nc.vector.max(m8a[:], sc[:])
                    scr = mid1.tile([128, NN], F32, tag="scr")
                    nc.vector.match_replace(scr[:], m8a[:], sc[:], -1e30)
                    m8b = small.tile([128, 8], F32, tag="m8b")
                    nc.vector.max(m8b[:], scr[:])
                    negm1 = small.tile([128, 1], F32, tag="negm1")
                    nc.vector.tensor_scalar_mul(negm1[:], m8a[:, 0:1], -1.0)
                    esc = mid1.tile([128, NN], F32, tag="esc")
                    nc.scalar.activation(esc[:], sc[:], AF.Exp, bias=negm1[:])
                    w_sb = mid1.tile([128, NN], F32, tag="w_sb")
                    zr = small.tile([128, 1], F32, tag="zr")
                    nc.vector.scalar_tensor_tensor(w_sb[:], sc[:], m8b[:, 7:8],
                                                   esc[:], op0=ALU.is_ge,
                                                   op1=ALU.mult, accum_out=zr[:])
                    rzr = small.tile([128, 1], F32, tag="rzr")
                    nc.vector.reciprocal(rzr[:], zr[:])
                    if DEBUG_TAPS and l == 0:
                        nc.sync.dma_start(d_tap.ap()[6 + t, :, :], w_sb[:])
                    wT = act.tile([128, DC, 128], F32, tag="wT")
                    for c in range(DC):
                        transpose_to(w_sb[:, c * 128:(c + 1) * 128],
                                     wT[:, c, :], use_act=(c % 2 == 0))
                    dl_ps = psA.tile([128, D], F32, tag="mm1")
                    for c in range(DC):
                        nc.tensor.matmul(dl_ps[:], wT[:, c, :], nemb_sb[:, c, :],
                                         start=(c == 0), stop=(c == DC - 1))
                    tc_ps = psA.tile([128, NB], F32, tag="mm1")
                    for c in range(DC):
                        nc.tensor.matmul(tc_ps[:], wT[:, c, :],
                                         coords_sb[:, c, :],
                                         start=(c == 0), stop=(c == DC - 1))
                    nc.vector.scalar_tensor_tensor(x_sb[t][:], dl_ps[:], rzr[:],
                                                   x_sb[t][:], op0=ALU.mult,
                                                   op1=ALU.add)
                    tc_sb = small.tile([128, NB], F32, tag="tc_sb")
                    nc.vector.tensor_scalar_mul(tc_sb[:], tc_ps[:], rzr[:])

                    n2_t = _layernorm(nc, pools, x_sb[t][:])
                    n2T = act.tile([128, DC, 128], F32, tag="n2T")
                    for c in range(DC):
                        transpose_to(n2_t[:, c * 128:(c + 1) * 128],
                                     n2T[:, c, :], use_act=(c % 2 == 1))
                    ha_ps = psA.tile([128, NBR], F32, tag="mm1")
                    for c in range(DC):
                        nc.tensor.matmul(ha_ps[:], n2T[:, c, :],
                                         aflat_sb[:, c, :],
                                         start=(c == 0), stop=(c == DC - 1))
                    hm = mid1.tile([128, NB, R], F32, tag="hm")
                    nc.vector.tensor_tensor(
                        hm[:], ha_ps[:].rearrange("p (n r) -> p n r", n=NB),
                        tc_sb[:, :, None].broadcast_to([128, NB, R]),
                        op=ALU.mult)
                    h_t = small.tile([128, R], F32, tag="h_t")
                    nc.vector.tensor_reduce(
                        h_t[:], hm[:].rearrange("p n r -> p r n"), axis=AX.X,
                        op=ALU.add)
                    hp = mid1.tile([128, NB, R], F32, tag="hp")
                    nc.vector.tensor_tensor(
                        hp[:], h_t[:, None, :].broadcast_to([128, NB, R]),
                        tc_sb[:, :, None].broadcast_to([128, NB, R]),
                        op=ALU.mult)
                    hpT = act.tile([128, DC, 128], F32, tag="hpT")
                    hpf = hp[:].rearrange("p n r -> p (n r)")
                    for c in range(DC):
                        transpose_to(hpf[:, c * 128:(c + 1) * 128],
                                     hpT[:, c, :], use_act=(c % 2 == 0))
                    hf_ps = psC.tile([128, DF], F32, tag="big")
                    for c in range(DC):
                        for f in range(DF // 512):
                            nc.tensor.matmul(
                                hf_ps[:, f * 512:(f + 1) * 512], hpT[:, c, :],
                                bflat_sb[:, c, f * 512:(f + 1) * 512],
                                start=(c == 0), stop=(c == DC - 1))
                    out_ps = psA.tile([128, D], F32, tag="mm1")
                    for fc4 in range(DF // 512):
                        fsl = slice(fc4 * 512, (fc4 + 1) * 512)
                        ef = actD.tile([128, 512], F32, tag="ef")
                        nc.scalar.activation(ef[:], hf_ps[:, fsl], AF.Erf,
                                             scale=0.7071067811865476)
                        gl = actD.tile([128, 512], F32, tag="gl")
                        nc.vector.scalar_tensor_tensor(gl[:], ef[:], 1.0,
                                                       hf_ps[:, fsl],
                                                       op0=ALU.add, op1=ALU.mult)
                        if DEBUG_TAPS and l == 0 and fc4 == 0:
                            nc.sync.dma_start(d_tap.ap()[10 + t, :, :], gl[:])
                        for cc in range(4):
                            c16 = fc4 * 4 + cc
                            glT = actD.tile([128, 128], F32, tag="glT")
                            transpose_to(gl[:, cc * 128:(cc + 1) * 128],
                                         glT[:], use_act=(cc % 2 == 0))
                            nc.tensor.matmul(out_ps[:], glT[:],
                                             dwwh_sb[:, c16, :],
                                             start=(c16 == 0),
                                             stop=(c16 == FC - 1))
                    nc.vector.tensor_add(x_sb[t][:], out_ps[:], x_sb[t][:])
                    if DEBUG_TAPS and l in (0, 1, 3):
                        slot = {0: 8, 1: 12, 3: 14}[l]
                        nc.sync.dma_start(d_tap.ap()[slot + t, :, :], x_sb[t][:])

            # ---- final LN -> transposed AllGather(8) -> logits ----
            xfT = act.tile([128, DC, CHUNK], BF16, tag="kTo")
            for t in range(NT):
                xf_t = _layernorm(nc, pools, x_sb[t][:])
                for c in range(DC):
                    transpose_to(xf_t[:, c * 128:(c + 1) * 128],
                                 xfT[:, c, t * 128:(t + 1) * 128],
                                 use_act=(c % 2 == 0))
            for c in range(DC):
                nc.sync.dma_start(ag2_in.ap()[c * 128:(c + 1) * 128, :],
                                  xfT[:, c, :])
            nc.gpsimd.collective_compute(
                "AllGather", ALU.bypass, replica_groups=ALL_GROUP,
                ins=[ag2_in.ap().opt()], outs=[ag2_out.ap().opt()])

            xfT_all = wts.tile([128, DC, B * S], BF16, tag="dwwh")
            for rr in range(NCORES):
                for c in range(DC):
                    nc.sync.dma_start(
                        xfT_all[:, c, rr * CHUNK:(rr + 1) * CHUNK],
                        ag2_out.ap()[rr * D + c * 128:rr * D + (c + 1) * 128, :])
            for vh in range(2):
                embT_sb = wts.tile([128, DC, VHALF], BF16, tag="bflat")
                for c in range(DC):
                    nc.sync.dma_start(
                        embT_sb[:, c, :],
                        d_embT.ap()[c * 128:(c + 1) * 128,
                                    vh * VHALF:(vh + 1) * VHALF])
                for tt in range(B * S // 128):
                    for vc in range(VHALF // VCH):
                        lp = psA.tile([128, VCH], F32, tag="mm1")
                        for c in range(DC):
                            nc.tensor.matmul(
                                lp[:],
                                xfT_all[:, c, tt * 128:(tt + 1) * 128],
                                embT_sb[:, c, vc * VCH:(vc + 1) * VCH],
                                start=(c == 0), stop=(c == DC - 1))
                        ot = attp.tile([128, VCH], F32, tag="pe")
                        if (tt + vc) % 2 == 0:
                            nc.scalar.copy(ot[:], lp[:])
                        else:
                            nc.vector.tensor_copy(ot[:], lp[:])
                        vco = vh * VHALF + vc * VCH
                        nc.sync.dma_start(
                            d_out.ap()[tt * 128:(tt + 1) * 128,
                                       vco:vco + VCH], ot[:])

    nc.compile()
    _cache["built"] = nc
    return nc


def _prep_inputs(inputs):
    def f32(x):
        return np.ascontiguousarray(np.asarray(x), dtype=np.float32)

    for name in ("qb", "kb", "vb", "gate_b", "n1b", "n2b", "nf_b", "dw_b"):
        if np.abs(np.asarray(inputs[name])).max() != 0:
            raise NotImplementedError(f"nonzero bias {name} unsupported")
    for name in ("n1s", "n2s", "nf_s"):
        if not np.all(np.asarray(inputs[name]) == 1.0):
            raise NotImplementedError(f"non-unit scale {name} unsupported")

    ids = np.asarray(inputs["input_ids"]).astype(np.int64)
    emb = f32(inputs["token_emb"])
    pos = f32(inputs["pos_emb"])
    shared = {
        "emb": emb,
        "ident": np.eye(128, dtype=np.float32),
        "qw": f32(inputs["qw"]),
        "kw": f32(inputs["kw"]),
        "vw": f32(inputs["vw"]),
        "nembT": np.ascontiguousarray(f32(inputs["neuron_emb"]).transpose(0, 2, 1)),
        "nemb": f32(inputs["neuron_emb"]),
        "coords": f32(inputs["neuron_coords"]),
        "gw1": np.ascontiguousarray(f32(inputs["gate_w"])[:, :D, :]),
        "gw2": np.ascontiguousarray(f32(inputs["gate_w"])[:, D:, :]),
        "aflat": np.ascontiguousarray(
            f32(inputs["basis_A"]).transpose(0, 2, 1, 3).reshape(L, D, NBR)),
        "bflat": np.ascontiguousarray(f32(inputs["basis_B"]).reshape(L, NBR, DF)),
        "dwwh": np.ascontiguousarray(0.5 * f32(inputs["dw_w"])),
    }
    in_maps = []
    for core in range(NCORES):
        b, ch = divmod(core, GROUP)
        q0 = ch * CHUNK
        cid = ids[b, q0:q0 + CHUNK]
        idx16 = np.zeros((16, 16), np.int16)
        for i in range(CHUNK):
            idx16[i % 16, i // 16] = cid[i]
        p = np.arange(128)[:, None]
        j = np.arange(CHUNK)[None, :]
        m = np.zeros((128, KC, CHUNK), np.float32)
        for kc in range(KC):
            m[:, kc, :] = ((q0 + j) >= (kc * 128 + p)).astype(np.float32)
        in_maps.append({
            **shared,
            "idx16": np.tile(idx16, (8, 1)),
            "pos": np.ascontiguousarray(
                pos[q0:q0 + CHUNK].reshape(NT, 128, D).transpose(1, 0, 2)),
            "mask01": m,
            "embTs": np.ascontiguousarray(
                emb[core * VS:(core + 1) * VS].T.astype(ml_dtypes.bfloat16)),
        })
    return in_maps


def _run_sim(nc, in_maps):
    import scipy.special
    import concourse.bass_interp as bi
    import concourse.mybir as mb

    orig = bi.InstructionExecutor.visit_InstActivation

    def patched(self, instruction, *, reg_snapshot=None):
        if instruction.func == mb.ActivationFunctionType.Sqrt:
            inp = self.view_ap(instruction.ins[0], bi.Direction.READ,
                               instruction, reg_snapshot=reg_snapshot)
            arr = np.asarray(inp, np.float32)
            if not np.all(arr >= 0) or not np.all(np.isfinite(arr)):
                print(f"BAD SQRT INPUT at {instruction.name}: "
                      f"min={np.nanmin(arr)} nan={np.isnan(arr).sum()}")
            out = self.view_ap(instruction.outs[0], bi.Direction.WRITE,
                               instruction, reg_snapshot=reg_snapshot)
            out[:] = np.sqrt(np.maximum(arr, 0)).reshape(out.shape)
            return
        if instruction.func == mb.ActivationFunctionType.Erf:
            inp = self.view_ap(instruction.ins[0], bi.Direction.READ,
                               instruction, reg_snapshot=reg_snapshot)
            scale = instruction.ins[2]
            sval = scale.value if isinstance(scale, mb.ImmediateValue) else 1.0
            out = self.view_ap(instruction.outs[0], bi.Direction.WRITE,
                               instruction, reg_snapshot=reg_snapshot)
            arr = np.asarray(inp, np.float32).reshape(inp.shape[0], -1)
            res = scipy.special.erf(arr * np.float32(sval)).astype(np.float32)
            out[:] = res.reshape(out.shape)
            return
        return orig(self, instruction, reg_snapshot=reg_snapshot)

    bi.InstructionExecutor.visit_InstActivation = patched
    try:
        sim = bi.MultiCoreSim(nc, num_cores=NCORES, trace=False,
                              require_finite=False, require_nnan=False)
        for i, core in sim.cores.items():
            for name, arr in in_maps[i].items():
                core.tensor(name)[:] = arr
        sim.simulate(check_with_hw=False)
        outs = []
        for i in range(NCORES):
            d = {"out": np.array(sim.cores[i].mem_tensor("out"))}
            if DEBUG_TAPS:
                d["tap"] = np.array(sim.cores[i].mem_tensor("tap"))
            outs.append(d)
        return outs
    finally:
        bi.InstructionExecutor.visit_InstActivation = orig


def kernel(**inputs):
    global LAST_EXEC_NS
    nc = _build()
    in_maps = _prep_inputs(inputs)
    if SIM:
        results = _run_sim(nc, in_maps)
    else:
        from concourse.bass_utils import run_bass_kernel_spmd
        if TRACE:
            import concourse.bass_utils as bass_utils
            bass_utils.upload_artifacts = lambda tmpdir: str(tmpdir)
        res = run_bass_kernel_spmd(nc, in_maps, core_ids=list(range(NCORES)),
                                   trace=TRACE)
        LAST_EXEC_NS = res.exec_time_ns
        results = res.results
    full = np.concatenate([results[i]["out"] for i in range(NCORES)], axis=1)
    return full.reshape(B, S, V)


# revision 14
# speedup vs baseline: 1.0208x; 1.0208x over previous
"""Trainium2 Bass kernel for nn_DAWN_87677462380612 (moe_routing).

Sharding: 8 cores = 2 batches x 4 sequence chunks of 256 tokens.
Per layer: each core computes LN1/Q/K/V for its own 256 tokens, AllGathers
(K feature-major, V token-major) within its batch group of 4 cores, runs
causal attention for its queries against all 1024 keys (transposed-score
formulation: S_T[k,q] so the exp'd probs serve directly as AV's lhsT, with
a ones-column in V producing the softmax denominator), computes routing
scores, exact top-16 via vector max8 + match_replace + max8 (threshold
trick turns the top-k gather into dense masked-softmax GEMMs), then the
basis-coordinate MLP. Final LN locally -> AllGather(all 8, transposed) ->
vocab-parallel logits GEMM (4000 vocab columns per core) in float32r.

All trunk GEMMs run in fp32 (the reference top-16 selection has score gaps
down to 2.3e-6; bf16 anywhere flips selections and produces O(1) logit
errors). Softmax exp skips max-subtraction (|logit| <= 1.3). Gelu is exact
via the Erf LUT with the 0.5 factor folded into dw_w host-side.
"""
import ml_dtypes
import numpy as np

import concourse.bacc as bacc
import concourse.bass as bass
import concourse.mybir as mybir
import concourse.tile as tile

F32 = mybir.dt.float32
F32R = mybir.dt.float32r
BF16 = mybir.dt.bfloat16
I16 = mybir.dt.int16
AF = mybir.ActivationFunctionType
ALU = mybir.AluOpType
AX = mybir.AxisListType

B, S, V, D, DF, L, H, NN, K, NB, R = 2, 1024, 32000, 512, 2048, 4, 8, 512, 16, 8, 64
NCORES, GROUP, CHUNK = 8, 4, 256
VS = V // NCORES          # 4000
DH = D // H               # 64
NT = CHUNK // 128         # 2 token tiles per core
KC = S // 128             # 8 key chunks
DC = D // 128             # 4 feature chunks
FC = DF // 128            # 16
NBR = NB * R              # 512
VCH = 500                 # vocab chunk
VHALF = VS // 2           # 2000 (embT loaded in halves)
EPS = 1e-5

SIM = False
TRACE = False
DEBUG_TAPS = False
LAST_EXEC_NS = None

_cache = {}


def _rsqrt(nc, small, v):
    """Accurate 1/sqrt(v): sqrt LUT + reciprocal + 2 Newton steps."""
    sq = small.tile([128, 1], F32, tag="rs0")
    nc.scalar.sqrt(sq[:], v[:])
    r = small.tile([128, 1], F32, tag="rs1")
    nc.vector.reciprocal(r[:], sq[:])
    for i in range(2):
        a = small.tile([128, 1], F32, tag=f"rs2_{i}")
        nc.vector.tensor_mul(a[:], r[:], r[:])
        bq = small.tile([128, 1], F32, tag=f"rs3_{i}")
        nc.vector.tensor_mul(bq[:], a[:], v[:])
        d = small.tile([128, 1], F32, tag=f"rs4_{i}")
        nc.vector.tensor_scalar(d[:], bq[:], -0.5, 1.5, op0=ALU.mult, op1=ALU.add)
        r2 = small.tile([128, 1], F32, tag=f"rs5_{i}")
        nc.vector.tensor_mul(r2[:], r[:], d[:])
        r = r2
    return r


def _layernorm(nc, pools, x_ap):
    """LN over free axis (D=512), unit scale / zero bias. Returns [128, D]."""
    act, small = pools["act"], pools["small"]
    s = small.tile([128, 1], F32, tag="ln_s")
    nc.vector.reduce_sum(s[:], x_ap, axis=AX.X)
    nm = small.tile([128, 1], F32, tag="ln_nm")
    nc.vector.tensor_scalar_mul(nm[:], s[:], -1.0 / D)
    ct = act.tile([128, D], F32, tag="ln_c")
    nc.vector.tensor_scalar_add(ct[:], x_ap, nm[:])
    sq = act.tile([128, D], F32, tag="ctx")  # junk buffer, reuse ctx slot
    ss = small.tile([128, 1], F32, tag="ln_ss")
    nc.scalar.activation(sq[:], ct[:], AF.Square, accum_out=ss[:])
    v = small.tile([128, 1], F32, tag="ln_v")
    nc.vector.tensor_scalar(v[:], ss[:], 1.0 / D, EPS, op0=ALU.mult, op1=ALU.add)
    r = _rsqrt(nc, small, v)
    out = act.tile([128, D], F32, tag="ln_o")
    nc.vector.tensor_scalar_mul(out[:], ct[:], r[:])
    return out


def _build():
    if "built" in _cache:
        return _cache["built"]
    import concourse.tile_utils as tile_utils
    tile_utils.max_sbuf_usage = 207 * 1024  # cayman: 208KB usable/partition

    nc = bacc.Bacc("TRN2", target_bir_lowering=False, debug=False,
                   num_devices=NCORES)

    d_idx = nc.dram_tensor("idx16", [128, 16], I16, kind="ExternalInput")
    d_emb = nc.dram_tensor("emb", [V, D], F32, kind="ExternalInput")
    d_pos = nc.dram_tensor("pos", [128, NT, D], F32, kind="ExternalInput")
    d_mask = nc.dram_tensor("mask01", [128, KC, CHUNK], F32, kind="ExternalInput")
    d_ident = nc.dram_tensor("ident", [128, 128], F32, kind="ExternalInput")
    d_qw = nc.dram_tensor("qw", [L, D, D], F32, kind="ExternalInput")
    d_kw = nc.dram_tensor("kw", [L, D, D], F32, kind="ExternalInput")
    d_vw = nc.dram_tensor("vw", [L, D, D], F32, kind="ExternalInput")
    d_nembT = nc.dram_tensor("nembT", [L, D, NN], F32, kind="ExternalInput")
    d_nemb = nc.dram_tensor("nemb", [L, NN, D], F32, kind="ExternalInput")
    d_coords = nc.dram_tensor("coords", [L, NN, NB], F32, kind="ExternalInput")
    d_gw1 = nc.dram_tensor("gw1", [L, D, 2], F32, kind="ExternalInput")
    d_gw2 = nc.dram_tensor("gw2", [L, D, 2], F32, kind="ExternalInput")
    d_aflat = nc.dram_tensor("aflat", [L, D, NBR], F32, kind="ExternalInput")
    d_bflat = nc.dram_tensor("bflat", [L, NBR, DF], F32, kind="ExternalInput")
    d_dwwh = nc.dram_tensor("dwwh", [L, DF, D], F32, kind="ExternalInput")
    d_embT = nc.dram_tensor("embTs", [D, VS], BF16, kind="ExternalInput")
    d_out = nc.dram_tensor("out", [B * S, VS], F32, kind="ExternalOutput")
    d_tap = None
    if DEBUG_TAPS:
        d_tap = nc.dram_tensor("tap", [16, 128, NN], F32, kind="ExternalOutput")

    kv_in = nc.dram_tensor("kv_in", [S, CHUNK], F32)
    kv_out = nc.dram_tensor("kv_out", [GROUP * S, CHUNK], F32)
    ag2_in = nc.dram_tensor("ag2_in", [D, CHUNK], BF16)
    ag2_out = nc.dram_tensor("ag2_out", [NCORES * D, CHUNK], BF16,
                             addr_space="Shared")
    KV_GROUPS = [[0, 1, 2, 3], [4, 5, 6, 7]]
    ALL_GROUP = [list(range(NCORES))]

    with tile.TileContext(nc) as tc:
        with (
            tc.tile_pool(name="persist", bufs=1) as persist,
            tc.tile_pool(name="wts", bufs=1) as wts,
            tc.tile_pool(name="act", bufs=1) as act,
            tc.tile_pool(name="actD", bufs=2) as actD,
            tc.tile_pool(name="attp", bufs=3) as attp,
            tc.tile_pool(name="mid1", bufs=1) as mid1,
            tc.tile_pool(name="mid2", bufs=1) as mid2,
            tc.tile_pool(name="small", bufs=2) as small,
            tc.tile_pool(name="psA", bufs=2, space="PSUM") as psA,
            tc.tile_pool(name="psB", bufs=2, space="PSUM") as psB,
            tc.tile_pool(name="psC", bufs=1, space="PSUM") as psC,
        ):
            pools = {"act": act, "small": small}

            def transpose_to(src_ap, dst_ap, use_act):
                pt = psB.tile([128, 128], F32, tag="tr")
                nc.tensor.transpose(pt[:], src_ap, ident[:])
                if use_act:
                    nc.scalar.copy(dst_ap, pt[:])
                else:
                    nc.vector.tensor_copy(dst_ap, pt[:])

            ident = persist.tile([128, 128], F32, tag="ident")
            nc.sync.dma_start(ident[:], d_ident.ap())
            mask = persist.tile([128, KC, CHUNK], F32, tag="mask")
            nc.sync.dma_start(mask[:], d_mask.ap())

            # ---- embedding gather + positional ----
            idx = persist.tile([128, 16], I16, tag="idx")
            nc.sync.dma_start(idx[:], d_idx.ap())
            xg = act.tile([128, NT, D], F32, tag="qT")
            nc.gpsimd.dma_gather(xg[:], d_emb.ap(), idx[:], num_idxs=CHUNK,
                                 num_idxs_reg=CHUNK, elem_size=D)
            pos = act.tile([128, NT, D], F32, tag="kTo")
            nc.sync.dma_start(pos[:], d_pos.ap())
            x_sb = []
            for t in range(NT):
                xt = persist.tile([128, D], F32, tag=f"x{t}")
                nc.vector.tensor_add(xt[:], xg[:, t, :], pos[:, t, :])
                x_sb.append(xt)

            for l in range(L):
                # ---- layer weights (single-buffered slots, streamed) ----
                qw_sb = wts.tile([128, DC, D], F32, tag="qw")
                kw_sb = wts.tile([128, DC, D], F32, tag="kw")
                vw_sb = wts.tile([128, DC, D], F32, tag="vw")
                nembT_sb = wts.tile([128, DC, NN], F32, tag="nembT")
                nemb_sb = wts.tile([128, DC, D], F32, tag="nemb")
                coords_sb = wts.tile([128, DC, NB], F32, tag="coords")
                gw1_sb = wts.tile([128, DC, 2], F32, tag="gw1")
                gw2_sb = wts.tile([128, DC, 2], F32, tag="gw2")
                aflat_sb = wts.tile([128, DC, NBR], F32, tag="aflat")
                bflat_sb = wts.tile([128, DC, DF], F32, tag="bflat")
                for c in range(DC):
                    rows = slice(c * 128, (c + 1) * 128)
                    nc.sync.dma_start(qw_sb[:, c, :], d_qw.ap()[l, rows, :])
                    nc.sync.dma_start(kw_sb[:, c, :], d_kw.ap()[l, rows, :])
                    nc.sync.dma_start(vw_sb[:, c, :], d_vw.ap()[l, rows, :])
                    nc.sync.dma_start(nembT_sb[:, c, :], d_nembT.ap()[l, rows, :])
                    nc.sync.dma_start(nemb_sb[:, c, :], d_nemb.ap()[l, rows, :])
                    nc.sync.dma_start(coords_sb[:, c, :], d_coords.ap()[l, rows, :])
                    nc.sync.dma_start(gw1_sb[:, c, :], d_gw1.ap()[l, rows, :])
                    nc.sync.dma_start(gw2_sb[:, c, :], d_gw2.ap()[l, rows, :])
                    nc.sync.dma_start(aflat_sb[:, c, :], d_aflat.ap()[l, rows, :])
                    nc.sync.dma_start(bflat_sb[:, c, :], d_bflat.ap()[l, rows, :])
                dwwh_sb = wts.tile([128, FC, D], F32, tag="dwwh")
                for c in range(FC):
                    nc.sync.dma_start(dwwh_sb[:, c, :],
                                      d_dwwh.ap()[l, c * 128:(c + 1) * 128, :])

                # ---- LN1 + feature-major transpose ----
                n1T = act.tile([128, DC, CHUNK], F32, tag="n1T")
                for t in range(NT):
                    n1_t = _layernorm(nc, pools, x_sb[t][:])
                    if DEBUG_TAPS and l == 0:
                        nc.sync.dma_start(d_tap.ap()[0 + t, :, :], n1_t[:])
                    for c in range(DC):
                        transpose_to(n1_t[:, c * 128:(c + 1) * 128],
                                     n1T[:, c, t * 128:(t + 1) * 128],
                                     use_act=(c % 2 == 0))

                # ---- Q,K feature-major / V token-major (own tokens) ----
                qT = act.tile([128, DC, CHUNK], F32, tag="qT")
                kTo = act.tile([128, DC, CHUNK], F32, tag="kTo")
                for o in range(DC):
                    for wsb, dst in ((qw_sb, qT), (kw_sb, kTo)):
                        pp = psA.tile([128, CHUNK], F32, tag="mm1")
                        for c in range(DC):
                            nc.tensor.matmul(pp[:],
                                             wsb[:, c, o * 128:(o + 1) * 128],
                                             n1T[:, c, :],
                                             start=(c == 0), stop=(c == DC - 1))
                        nc.scalar.copy(dst[:, o, :], pp[:])
                for c in range(DC):
                    nc.sync.dma_start(kv_in.ap()[c * 128:(c + 1) * 128, :],
                                      kTo[:, c, :])
                for t in range(NT):
                    pp = psA.tile([128, D], F32, tag="mm1")
                    for c in range(DC):
                        nc.tensor.matmul(pp[:], n1T[:, c, t * 128:(t + 1) * 128],
                                         vw_sb[:, c, :],
                                         start=(c == 0), stop=(c == DC - 1))
                    vt = actD.tile([128, D], F32, tag="v_own")
                    nc.vector.tensor_copy(vt[:], pp[:])
                    dst = kv_in.ap()[D + t * 256:D + (t + 1) * 256, :].rearrange(
                        "(p two) f -> p (two f)", two=2)
                    nc.sync.dma_start(dst, vt[:])
                nc.gpsimd.collective_compute(
                    "AllGather", ALU.bypass, replica_groups=KV_GROUPS,
                    ins=[kv_in.ap().opt()], outs=[kv_out.ap().opt()])

                # token-scores GEMM needs no K/V: fill the AllGather bubble,
                # parking results in the LN scratch slots (idle until LN2)
                ts_sb = []
                for t in range(NT):
                    tp = psA.tile([128, NN], F32, tag="mm1")
                    for c in range(DC):
                        nc.tensor.matmul(tp[:], n1T[:, c, t * 128:(t + 1) * 128],
                                         nembT_sb[:, c, :],
                                         start=(c == 0), stop=(c == DC - 1))
                    tss = mid2.tile([128, NN], F32, tag=f"ts{t}")
                    nc.scalar.copy(tss[:], tp[:])
                    ts_sb.append(tss)

                # ---- attention: stream key-chunks ----
                # Per kc: single-shot AV matmuls into a 4-bank psum tile
                # (512B slot per (h,t) so each write stays inside one bank;
                # no cross-kc psum accumulation -- the 2KB zero-region start
                # semantics would discard sibling slots' data). DVE folds the
                # partials into an SBUF accumulator region-by-region so the
                # adds pipeline against the PE.
                ctxacc = act.tile([128, H, NT, DH + 1], F32, tag="ctxacc")
                for kc in range(KC):
                    r, th = divmod(kc, 2)
                    KTc = actD.tile([128, DC, 128], F32, tag="KTc")
                    for c in range(DC):
                        nc.sync.dma_start(
                            KTc[:, c, :],
                            kv_out.ap()[r * S + c * 128:r * S + (c + 1) * 128,
                                        th * 128:(th + 1) * 128])
                    Vgc = actD.tile([128, H, DH + 1], F32, tag="Vgc")
                    src = kv_out.ap()[r * S + D + th * 256:
                                      r * S + D + (th + 1) * 256, :]
                    src = src.rearrange("(p two) f -> p (two f)", two=2)
                    src = src.rearrange("p (h dh) -> p h dh", h=H)
                    nc.sync.dma_start(Vgc[:, :, 0:DH], src)
                    nc.vector.memset(Vgc[:, :, DH], 1.0)
                    ctx_ps = psC.tile([128, H, NT, 128], F32, tag="big")
                    for h in range(H):
                        po = (h % 2) * 64
                        ch = h // 2
                        st = psB.tile([128, CHUNK], F32, tag="tr")
                        nc.tensor.matmul(st[:], KTc[po:po + DH, ch, :],
                                         qT[po:po + DH, ch, :],
                                         start=True, stop=True)
                        pe = attp.tile([128, CHUNK], F32, tag="pe")
                        nc.scalar.activation(pe[:], st[:], AF.Exp, scale=0.125)
                        pm = attp.tile([128, CHUNK], F32, tag="pm")
                        nc.vector.tensor_mul(pm[:], pe[:], mask[:, kc, :])
                        for t in range(NT):
                            nc.tensor.matmul(
                                ctx_ps[:, h, t, 0:DH + 1],
                                pm[:, t * 128:(t + 1) * 128],
                                Vgc[:, h, :],
                                start=True, stop=True,
                                skip_group_check=True)
                        if h % 2 == 1:
                            hs = slice(h - 1, h + 1)
                            if kc == 0:
                                nc.vector.tensor_copy(
                                    ctxacc[:, hs, :, :],
                                    ctx_ps[:, hs, :, 0:DH + 1])
                            else:
                                nc.vector.tensor_add(
                                    ctxacc[:, hs, :, :], ctxacc[:, hs, :, :],
                                    ctx_ps[:, hs, :, 0:DH + 1])

                # ---- ctx normalize + transpose ----
                ctxnT = act.tile([128, DC, CHUNK], F32, tag="qT")
                for t in range(NT):
                    cx = act.tile([128, H, DH], F32, tag="ctx")
                    nc.scalar.copy(cx[:], ctxacc[:, :, t, 0:DH])
                    z = small.tile([128, H], F32, tag="z")
                    nc.vector.tensor_copy(z[:], ctxacc[:, :, t, DH])
                    rz = small.tile([128, H], F32, tag="rz")
                    nc.vector.reciprocal(rz[:], z[:])
                    cn = act.tile([128, H, DH], F32, tag="ctxn")
                    nc.vector.tensor_tensor(
                        cn[:], cx[:],
                        rz[:, :, None].broadcast_to([128, H, DH]), op=ALU.mult)
                    cnf = cn[:].rearrange("p h dh -> p (h dh)")
                    if DEBUG_TAPS and l == 0:
                        nc.sync.dma_start(d_tap.ap()[2 + t, :, :], cnf)
                    for c in range(DC):
                        transpose_to(cnf[:, c * 128:(c + 1) * 128],
                                     ctxnT[:, c, t * 128:(t + 1) * 128],
                                     use_act=(c % 2 == 1))

                # ---- routing + MLP per token tile ----
                for t in range(NT):
                    tsl = slice(t * 128, (t + 1) * 128)
                    cs_ps = psA.tile([128, NN], F32, tag="mm1")
                    for c in range(DC):
                        nc.tensor.matmul(cs_ps[:], ctxnT[:, c, tsl],
                                         nembT_sb[:, c, :],
                                         start=(c == 0), stop=(c == DC - 1))
                    gz_ps = psC.tile([128, 2], F32, tag="big")
                    for c in range(DC):
                        nc.tensor.matmul(gz_ps[:], n1T[:, c, tsl],
                                         gw1_sb[:, c, :],
                                         start=(c == 0), stop=False)
                        nc.tensor.matmul(gz_ps[:], ctxnT[:, c, tsl],
                                         gw2_sb[:, c, :],
                                         start=False, stop=(c == DC - 1))
                    gz_sb = small.tile([128, 2], F32, tag="gz_sb")
                    nc.scalar.copy(gz_sb[:], gz_ps[:])
                    zd = small.tile([128, 1], F32, tag="zd")
                    nc.vector.tensor_sub(zd[:], gz_sb[:, 1:2], gz_sb[:, 0:1])
                    en = small.tile([128, 1], F32, tag="en")
                    nc.scalar.activation(en[:], zd[:], AF.Exp, scale=-1.0)
                    sden = small.tile([128, 1], F32, tag="sden")
                    nc.vector.tensor_scalar_add(sden[:], en[:], 1.0)
                    g1 = small.tile([128, 1], F32, tag="g1")
                    nc.vector.reciprocal(g1[:], sden[:])
                    g0 = small.tile([128, 1], F32, tag="g0")
                    nc.vector.tensor_mul(g0[:], en[:], g1[:])

                    t1 = mid2.tile([128, NN], F32, tag="t1")
                    nc.vector.tensor_scalar_mul(t1[:], cs_ps[:], g1[:])
                    sc = mid2.tile([128, NN], F32, tag="sc")
                    nc.vector.scalar_tensor_tensor(sc[:], ts_sb[t][:], g0[:],
                                                   t1[:], op0=ALU.mult,
                                                   op1=ALU.add)
                    if DEBUG_TAPS and l == 0:
                        nc.sync.dma_start(d_tap.ap()[4 + t, :, :], sc[:])
                    m8a = small.tile([128, 8], F32, tag="m8a")
                    nc.vector.max(m8a[:], sc[:])
                    scr = mid1.tile([128, NN], F32, tag="scr")
                    nc.vector.match_replace(scr[:], m8a[:], sc[:], -1e30)
                    m8b = small.tile([128, 8], F32, tag="m8b")
                    nc.vector.max(m8b[:], scr[:])
                    negm1 = small.tile([128, 1], F32, tag="negm1")
                    nc.vector.tensor_scalar_mul(negm1[:], m8a[:, 0:1], -1.0)
                    esc = mid1.tile([128, NN], F32, tag="esc")
                    nc.scalar.activation(esc[:], sc[:], AF.Exp, bias=negm1[:])
                    w_sb = mid1.tile([128, NN], F32, tag="w_sb")
                    zr = small.tile([128, 1], F32, tag="zr")
                    nc.vector.scalar_tensor_tensor(w_sb[:], sc[:], m8b[:, 7:8],
                                                   esc[:], op0=ALU.is_ge,
                                                   op1=ALU.mult, accum_out=zr[:])
                    rzr = small.tile([128, 1], F32, tag="rzr")
                    nc.vector.reciprocal(rzr[:], zr[:])
                    if DEBUG_TAPS and l == 0:
                        nc.sync.dma_start(d_tap.ap()[6 + t, :, :], w_sb[:])
                    wT = act.tile([128, DC, 128], F32, tag="wT")
                    for c in range(DC):
                        transpose_to(w_sb[:, c * 128:(c + 1) * 128],
                                     wT[:, c, :], use_act=(c % 2 == 0))
                    dl_ps = psA.tile([128, D], F32, tag="mm1")
                    for c in range(DC):
                        nc.tensor.matmul(dl_ps[:], wT[:, c, :], nemb_sb[:, c, :],
                                         start=(c == 0), stop=(c == DC - 1))
                    tc_ps = psA.tile([128, NB], F32, tag="mm1")
                    for c in range(DC):
                        nc.tensor.matmul(tc_ps[:], wT[:, c, :],
                                         coords_sb[:, c, :],
                                         start=(c == 0), stop=(c == DC - 1))
                    nc.vector.scalar_tensor_tensor(x_sb[t][:], dl_ps[:], rzr[:],
                                                   x_sb[t][:], op0=ALU.mult,
                                                   op1=ALU.add)
                    tc_sb = small.tile([128, NB], F32, tag="tc_sb")
                    nc.vector.tensor_scalar_mul(tc_sb[:], tc_ps[:], rzr[:])

                    n2_t = _layernorm(nc, pools, x_sb[t][:])
                    n2T = act.tile([128, DC, 128], F32, tag="n2T")
                    for c in range(DC):
                        transpose_to(n2_t[:, c * 128:(c + 1) * 128],
                                     n2T[:, c, :], use_act=(c % 2 == 1))
                    ha_ps = psA.tile([128, NBR], F32, tag="mm1")
                    for c in range(DC):
                        nc.tensor.matmul(ha_ps[:], n2T[:, c, :],
                                         aflat_sb[:, c, :],
                                         start=(c == 0), stop=(c == DC - 1))
                    hm = mid1.tile([128, NB, R], F32, tag="hm")
                    nc.vector.tensor_tensor(
                        hm[:], ha_ps[:].rearrange("p (n r) -> p n r", n=NB),
                        tc_sb[:, :, None].broadcast_to([128, NB, R]),
                        op=ALU.mult)
                    h_t = small.tile([128, R], F32, tag="h_t")
                    nc.vector.tensor_reduce(
                        h_t[:], hm[:].rearrange("p n r -> p r n"), axis=AX.X,
                        op=ALU.add)
                    hp = mid1.tile([128, NB, R], F32, tag="hp")
                    nc.vector.tensor_tensor(
                        hp[:], h_t[:, None, :].broadcast_to([128, NB, R]),
                        tc_sb[:, :, None].broadcast_to([128, NB, R]),
                        op=ALU.mult)
                    hpT = act.tile([128, DC, 128], F32, tag="hpT")
                    hpf = hp[:].rearrange("p n r -> p (n r)")
                    for c in range(DC):
                        transpose_to(hpf[:, c * 128:(c + 1) * 128],
                                     hpT[:, c, :], use_act=(c % 2 == 0))
                    hf_ps = psC.tile([128, DF], F32, tag="big")
                    for c in range(DC):
                        for f in range(DF // 512):
                            nc.tensor.matmul(
                                hf_ps[:, f * 512:(f + 1) * 512], hpT[:, c, :],
                                bflat_sb[:, c, f * 512:(f + 1) * 512],
                                start=(c == 0), stop=(c == DC - 1))
                    out_ps = psA.tile([128, D], F32, tag="mm1")
                    for fc4 in range(DF // 512):
                        fsl = slice(fc4 * 512, (fc4 + 1) * 512)
                        ef = actD.tile([128, 512], F32, tag="ef")
                        nc.scalar.activation(ef[:], hf_ps[:, fsl], AF.Erf,
                                             scale=0.7071067811865476)
                        gl = actD.tile([128, 512], F32, tag="gl")
                        nc.vector.scalar_tensor_tensor(gl[:], ef[:], 1.0,
                                                       hf_ps[:, fsl],
                                                       op0=ALU.add, op1=ALU.mult)
                        if DEBUG_TAPS and l == 0 and fc4 == 0:
                            nc.sync.dma_start(d_tap.ap()[10 + t, :, :], gl[:])
                        for cc in range(4):
                            c16 = fc4 * 4 + cc
                            glT = actD.tile([128, 128], F32, tag="glT")
                            transpose_to(gl[:, cc * 128:(cc + 1) * 128],
                                         glT[:], use_act=(cc % 2 == 0))
                            nc.tensor.matmul(out_ps[:], glT[:],
                                             dwwh_sb[:, c16, :],
                                             start=(c16 == 0),
                                             stop=(c16 == FC - 1))
                    nc.vector.tensor_add(x_sb[t][:], out_ps[:], x_sb[t][:])
                    if DEBUG_TAPS and l in (0, 1, 3):
                        slot = {0: 8, 1: 12, 3: 14}[l]
                        nc.sync.dma_start(d_tap.ap()[slot + t, :, :], x_sb[t][:])

            # ---- final LN -> transposed AllGather(8) -> logits ----
            xfT = act.tile([128, DC, CHUNK], BF16, tag="kTo")
            for t in range(NT):
                xf_t = _layernorm(nc, pools, x_sb[t][:])
                for c in range(DC):
                    transpose_to(xf_t[:, c * 128:(c + 1) * 128],
                                 xfT[:, c, t * 128:(t + 1) * 128],
                                 use_act=(c % 2 == 0))
            for c in range(DC):
                nc.sync.dma_start(ag2_in.ap()[c * 128:(c + 1) * 128, :],
                                  xfT[:, c, :])
            nc.gpsimd.collective_compute(
                "AllGather", ALU.bypass, replica_groups=ALL_GROUP,
                ins=[ag2_in.ap().opt()], outs=[ag2_out.ap().opt()])

            xfT_all = wts.tile([128, DC, B * S], BF16, tag="dwwh")
            for rr in range(NCORES):
                for c in range(DC):
                    nc.sync.dma_start(
                        xfT_all[:, c, rr * CHUNK:(rr + 1) * CHUNK],
                        ag2_out.ap()[rr * D + c * 128:rr * D + (c + 1) * 128, :])
            for vh in range(2):
                embT_sb = wts.tile([128, DC, VHALF], BF16, tag="bflat")
                for c in range(DC):
                    nc.sync.dma_start(
                        embT_sb[:, c, :],
                        d_embT.ap()[c * 128:(c + 1) * 128,
                                    vh * VHALF:(vh + 1) * VHALF])
                for tt in range(B * S // 128):
                    for vc in range(VHALF // VCH):
                        lp = psA.tile([128, VCH], F32, tag="mm1")
                        for c in range(DC):
                            nc.tensor.matmul(
                                lp[:],
                                xfT_all[:, c, tt * 128:(tt + 1) * 128],
                                embT_sb[:, c, vc * VCH:(vc + 1) * VCH],
                                start=(c == 0), stop=(c == DC - 1))
                        ot = attp.tile([128, VCH], F32, tag="pe")
                        if (tt + vc) % 2 == 0:
                            nc.scalar.copy(ot[:], lp[:])
                        else:
                            nc.vector.tensor_copy(ot[:], lp[:])
                        vco = vh * VHALF + vc * VCH
                        nc.sync.dma_start(
                            d_out.ap()[tt * 128:(tt + 1) * 128,
                                       vco:vco + VCH], ot[:])

    nc.compile()
    _cache["built"] = nc
    return nc


def _prep_inputs(inputs):
    def f32(x):
        return np.ascontiguousarray(np.asarray(x), dtype=np.float32)

    for name in ("qb", "kb", "vb", "gate_b", "n1b", "n2b", "nf_b", "dw_b"):
        if np.abs(np.asarray(inputs[name])).max() != 0:
            raise NotImplementedError(f"nonzero bias {name} unsupported")
    for name in ("n1s", "n2s", "nf_s"):
        if not np.all(np.asarray(inputs[name]) == 1.0):
            raise NotImplementedError(f"non-unit scale {name} unsupported")

    ids = np.asarray(inputs["input_ids"]).astype(np.int64)
    emb = f32(inputs["token_emb"])
    pos = f32(inputs["pos_emb"])
    shared = {
        "emb": emb,
        "ident": np.eye(128, dtype=np.float32),
        "qw": f32(inputs["qw"]),
        "kw": f32(inputs["kw"]),
        "vw": f32(inputs["vw"]),
        "nembT": np.ascontiguousarray(f32(inputs["neuron_emb"]).transpose(0, 2, 1)),
        "nemb": f32(inputs["neuron_emb"]),
        "coords": f32(inputs["neuron_coords"]),
        "gw1": np.ascontiguousarray(f32(inputs["gate_w"])[:, :D, :]),
        "gw2": np.ascontiguousarray(f32(inputs["gate_w"])[:, D:, :]),
        "aflat": np.ascontiguousarray(
            f32(inputs["basis_A"]).transpose(0, 2, 1, 3).reshape(L, D, NBR)),
        "bflat": np.ascontiguousarray(f32(inputs["basis_B"]).reshape(L, NBR, DF)),
        "dwwh": np.ascontiguousarray(0.5 * f32(inputs["dw_w"])),
    }
    in_maps = []
    for core in range(NCORES):
        b, ch = divmod(core, GROUP)
        q0 = ch * CHUNK
        cid = ids[b, q0:q0 + CHUNK]
        idx16 = np.zeros((16, 16), np.int16)
        for i in range(CHUNK):
            idx16[i % 16, i // 16] = cid[i]
        p = np.arange(128)[:, None]
        j = np.arange(CHUNK)[None, :]
        m = np.zeros((128, KC, CHUNK), np.float32)
        for kc in range(KC):
            m[:, kc, :] = ((q0 + j) >= (kc * 128 + p)).astype(np.float32)
        in_maps.append({
            **shared,
            "idx16": np.tile(idx16, (8, 1)),
            "pos": np.ascontiguousarray(
                pos[q0:q0 + CHUNK].reshape(NT, 128, D).transpose(1, 0, 2)),
            "mask01": m,
            "embTs": np.ascontiguousarray(
                emb[core * VS:(core + 1) * VS].T.astype(ml_dtypes.bfloat16)),
        })
    return in_maps


def _run_sim(nc, in_maps):
    import scipy.special
    import concourse.bass_interp as bi
    import concourse.mybir as mb

    orig = bi.InstructionExecutor.visit_InstActivation

    def patched(self, instruction, *, reg_snapshot=None):
        if instruction.func == mb.ActivationFunctionType.Sqrt:
            inp = self.view_ap(instruction.ins[0], bi.Direction.READ,
                               instruction, reg_snapshot=reg_snapshot)
            arr = np.asarray(inp, np.float32)
            if not np.all(arr >= 0) or not np.all(np.isfinite(arr)):
                print(f"BAD SQRT INPUT at {instruction.name}: "
                      f"min={np.nanmin(arr)} nan={np.isnan(arr).sum()}")
            out = self.view_ap(instruction.outs[0], bi.Direction.WRITE,
                               instruction, reg_snapshot=reg_snapshot)
            out[:] = np.sqrt(np.maximum(arr, 0)).reshape(out.shape)
            return
        if instruction.func == mb.ActivationFunctionType.Erf:
            inp = self.view_ap(instruction.ins[0], bi.Direction.READ,
                               instruction, reg_snapshot=reg_snapshot)
            scale = instruction.ins[2]
            sval = scale.value if isinstance(scale, mb.ImmediateValue) else 1.0
            out = self.view_ap(instruction.outs[0], bi.Direction.WRITE,
                               instruction, reg_snapshot=reg_snapshot)
            arr = np.asarray(inp, np.float32).reshape(inp.shape[0], -1)
            res = scipy.special.erf(arr * np.float32(sval)).astype(np.float32)
            out[:] = res.reshape(out.shape)
            return
        return orig(self, instruction, reg_snapshot=reg_snapshot)

    bi.InstructionExecutor.visit_InstActivation = patched
    try:
        sim = bi.MultiCoreSim(nc, num_cores=NCORES, trace=False,
                              require_finite=False, require_nnan=False)
        for i, core in sim.cores.items():
            for name, arr in in_maps[i].items():
                core.tensor(name)[:] = arr
        sim.simulate(check_with_hw=False)
        outs = []
        for i in range(NCORES):
            d = {"out": np.array(sim.cores[i].mem_tensor("out"))}
            if DEBUG_TAPS:
                d["tap"] = np.array(sim.cores[i].mem_tensor("tap"))
            outs.append(d)
        return outs
    finally:
        bi.InstructionExecutor.visit_InstActivation = orig


def kernel(**inputs):
    global LAST_EXEC_NS
    nc = _build()
    in_maps = _prep_inputs(inputs)
    if SIM:
        results = _run_sim(nc, in_maps)
    else:
        from concourse.bass_utils import run_bass_kernel_spmd
        if TRACE:
            import concourse.bass_utils as bass_utils
            bass_utils.upload_artifacts = lambda tmpdir: str(tmpdir)
        res = run_bass_kernel_spmd(nc, in_maps, core_ids=list(range(NCORES)),
                                   trace=TRACE)
        LAST_EXEC_NS = res.exec_time_ns
        results = res.results
    full = np.concatenate([results[i]["out"] for i in range(NCORES)], axis=1)
    return full.reshape(B, S, V)
